# revision 6
# baseline (speedup 1.0000x reference)
import math

import numpy as np

import concourse.bacc as bacc
import concourse.bass as bass
import concourse.mybir as mybir
import concourse.tile as tile
from concourse import bass_isa
from concourse.bass_utils import run_bass_kernel_spmd

B, S, H, NH, D = 2, 2048, 4096, 32, 128
N_CORES, DP, TP = 8, 2, 4
F = H // TP
ROPE_BASE = 10000.0

F32 = mybir.dt.float32
F32R = mybir.dt.float32r


def r(ap):
    return ap.bitcast(F32R)


def build_nc(s=S, h=H, f=F, slab_bytes=16 << 20):
    n_kc = h // 128
    n_fc = f // 128
    th_tok = min(s, max(128, slab_bytes // (4 * h)))
    n_th = s // th_tok
    tb = min(512, th_tok)
    n_tb = th_tok // tb
    qb = min(512, s)
    n_qb = s // qb
    n_sc = s // 128
    fb = min(512, f)
    n_fb = f // fb
    hb = min(512, h)
    n_hb = h // hb
    n_tc = th_tok // 128
    scale = 1.0 / math.sqrt(D)

    nc = bacc.Bacc("TRN2", target_bir_lowering=False)
    xT = nc.dram_tensor("xT", [h, s], F32R, kind="ExternalInput")
    wqT = nc.dram_tensor("wqT", [h, f], F32R, kind="ExternalInput")
    wkT = nc.dram_tensor("wkT", [h, f], F32R, kind="ExternalInput")
    wvT = nc.dram_tensor("wvT", [h, f], F32R, kind="ExternalInput")
    woT = nc.dram_tensor("woT", [f, h], F32R, kind="ExternalInput")
    cosT = nc.dram_tensor("cosT", [D, s], F32, kind="ExternalInput")
    sinT = nc.dram_tensor("sinT", [D, s], F32, kind="ExternalInput")
    pmat = nc.dram_tensor("pmat", [D, D], F32R, kind="ExternalInput")
    out = nc.dram_tensor("out", [s, h], F32, kind="ExternalOutput")

    xT_r = xT.rearrange("(kc p) t -> p kc t", p=128)
    wT_r = {
        "q": wqT.rearrange("(kc p) f -> p kc f", p=128),
        "k": wkT.rearrange("(kc p) f -> p kc f", p=128),
        "v": wvT.rearrange("(kc p) f -> p kc f", p=128),
    }

    with tile.TileContext(nc) as tc:
        with (
            tc.tile_pool(name="consts", bufs=1) as consts,
            tc.tile_pool(name="dram", bufs=1, space="DRAM") as dram,
        ):
            cos_t = consts.tile([D, s], F32, tag="cos")
            nc.sync.dma_start(out=cos_t[:, :], in_=cosT[:, :])
            sin_t = consts.tile([D, s], F32, tag="sin")
            nc.sync.dma_start(out=sin_t[:, :], in_=sinT[:, :])
            pm_t = consts.tile([D, D], F32R, tag="pm")
            nc.sync.dma_start(out=pm_t[:, :], in_=pmat[:, :])

            qT_d = dram.tile([f, s], F32R, tag="qT_d")
            kT_d = dram.tile([f, s], F32R, tag="kT_d")
            v_d = dram.tile([s, f], F32R, tag="v_d")
            ctxT_d = dram.tile([f, s], F32R, tag="ctxT_d")
            qkT_d = {"q": qT_d, "k": kT_d}

            for th in range(n_th):
                ts0 = th * th_tok
                with tc.tile_pool(name="xh", bufs=8) as xh_pool:
                    xh = []
                    kc_grp = max(1, n_kc // 8)
                    for g in range(0, n_kc, kc_grp):
                        xg = xh_pool.tile([128, kc_grp, th_tok], F32R, tag="xh")
                        nc.sync.dma_start(
                            out=xg[:, :, :],
                            in_=xT_r[:, g : g + kc_grp, ts0 : ts0 + th_tok],
                        )
                        xh.extend(xg[:, i, :] for i in range(kc_grp))

                    with (
                        tc.tile_pool(name="wqk", bufs=8) as wpool,
                        tc.tile_pool(name="ps_qk", bufs=4, space="PSUM") as ps_qk,
                        tc.tile_pool(name="ps_sh", bufs=2, space="PSUM") as ps_sh,
                        tc.tile_pool(name="ev_qk", bufs=3) as ev,
                    ):
                        for name in ("q", "k"):
                            for fc in range(n_fc):
                                pss = [
                                    ps_qk.tile([128, tb], F32, tag="ps_qk", name="ps_qk")
                                    for _ in range(n_tb)
                                ]
                                for kc in range(n_kc):
                                    wt = wpool.tile([128, 128], F32R, tag="w")
                                    nc.sync.dma_start(
                                        out=wt[:, :],
                                        in_=wT_r[name][
                                            :, kc, fc * 128 : (fc + 1) * 128
                                        ],
                                    )
                                    for t in range(n_tb):
                                        nc.tensor.matmul(
                                            pss[t][:, :],
                                            r(wt[:, :]),
                                            r(xh[kc][:, t * tb : (t + 1) * tb]),
                                            start=(kc == 0),
                                            stop=(kc == n_kc - 1),
                                        )
                                for t in range(n_tb):
                                    tsl = slice(ts0 + t * tb, ts0 + (t + 1) * tb)
                                    raw = ev.tile([128, tb], F32R, tag="raw")
                                    nc.scalar.copy(raw[:, :], pss[t][:, :])
                                    shp = ps_sh.tile([128, tb], F32, tag="ps_sh")
                                    nc.tensor.matmul(
                                        shp[:, :], r(pm_t[:, :]), r(raw[:, :]),
                                        start=True, stop=True,
                                    )
                                    m1 = ev.tile([128, tb], F32, tag="m1")
                                    nc.vector.tensor_mul(
                                        m1[:, :], raw[:, :], cos_t[:, tsl]
                                    )
                                    m2 = ev.tile([128, tb], F32, tag="m2")
                                    nc.vector.tensor_mul(
                                        m2[:, :], shp[:, :], sin_t[:, tsl]
                                    )
                                    qk = ev.tile([128, tb], F32R, tag="qk")
                                    nc.vector.tensor_add(qk[:, :], m1[:, :], m2[:, :])
                                    nc.sync.dma_start(
                                        out=qkT_d[name][
                                            fc * 128 : (fc + 1) * 128, tsl
                                        ],
                                        in_=qk[:, :],
                                    )

                    with (
                        tc.tile_pool(name="wv", bufs=4) as wvp,
                        tc.tile_pool(name="ps_v", bufs=8, space="PSUM") as ps_v,
                        tc.tile_pool(name="ev_v", bufs=4) as evv,
                    ):
                        for b in range(n_fb):
                            pss = [
                                ps_v.tile([128, fb], F32, tag="ps_v", name="ps_v")
                                for _ in range(n_tc)
                            ]
                            for kc in range(n_kc):
                                wt = wvp.tile([128, fb], F32R, tag="wv")
                                nc.sync.dma_start(
                                    out=wt[:, :],
                                    in_=wT_r["v"][:, kc, b * fb : (b + 1) * fb],
                                )
                                for t in range(n_tc):
                                    nc.tensor.matmul(
                                        pss[t][:, :],
                                        r(xh[kc][:, t * 128 : (t + 1) * 128]),
                                        r(wt[:, :]),
                                        start=(kc == 0),
                                        stop=(kc == n_kc - 1),
                                    )
                            for t in range(n_tc):
                                vo = evv.tile([128, fb], F32R, tag="vo")
                                nc.scalar.copy(vo[:, :], pss[t][:, :])
                                nc.sync.dma_start(
                                    out=v_d[
                                        ts0 + t * 128 : ts0 + (t + 1) * 128,
                                        b * fb : (b + 1) * fb,
                                    ],
                                    in_=vo[:, :],
                                )

            v_d_r = v_d.rearrange("(tc p) f -> p tc f", p=128)
            with (
                tc.tile_pool(name="qkv_h", bufs=2) as qkvp,
                tc.tile_pool(name="exps", bufs=2) as expp,
                tc.tile_pool(name="ps_s", bufs=4, space="PSUM") as ps_s,
                tc.tile_pool(name="ps_av", bufs=2, space="PSUM") as ps_av,
                tc.tile_pool(name="sm", bufs=4) as smp,
                tc.tile_pool(name="ctx", bufs=4) as ctxp,
            ):
                for hd in range(n_fc):
                    fsl = slice(hd * 128, (hd + 1) * 128)
                    q_h = qkvp.tile([128, s], F32R, tag="q_h")
                    nc.sync.dma_start(out=q_h[:, :], in_=qT_d[fsl, :])
                    k_h = qkvp.tile([128, s], F32R, tag="k_h")
                    nc.sync.dma_start(out=k_h[:, :], in_=kT_d[fsl, :])
                    v_h = qkvp.tile([128, n_sc, 128], F32R, tag="v_h")
                    nc.sync.dma_start(out=v_h[:, :, :], in_=v_d_r[:, :, fsl])

                    for q0 in range(n_qb):
                        qsl = slice(q0 * qb, (q0 + 1) * qb)
                        exps = expp.tile([128, n_sc, qb], F32R, tag="exps")
                        for kc in range(n_sc):
                            ps = ps_s.tile([128, qb], F32, tag="ps_s")
                            nc.tensor.matmul(
                                ps[:, :],
                                r(k_h[:, kc * 128 : (kc + 1) * 128]),
                                r(q_h[:, qsl]),
                                start=True, stop=True,
                            )
                            nc.scalar.activation(
                                exps[:, kc, :], ps[:, :],
                                mybir.ActivationFunctionType.Exp, scale=scale,
                            )
                        colsum = smp.tile([128, qb], F32, tag="colsum")
                        nc.vector.tensor_reduce(
                            colsum[:, :],
                            exps.rearrange("p c q -> p q c"),
                            axis=mybir.AxisListType.X,
                            op=mybir.AluOpType.add,
                        )
                        sums = smp.tile([128, qb], F32, tag="sums")
                        nc.gpsimd.partition_all_reduce(
                            sums[:, :], colsum[:, :], channels=128,
                            reduce_op=bass_isa.ReduceOp.add,
                        )
                        recip = smp.tile([128, qb], F32, tag="recip")
                        nc.vector.reciprocal(recip[:, :], sums[:, :])
                        psa = ps_av.tile([128, qb], F32, tag="ps_av")
                        for kc in range(n_sc):
                            nc.tensor.matmul(
                                psa[:, :],
                                r(v_h[:, kc, :]),
                                r(exps[:, kc, :]),
                                start=(kc == 0),
                                stop=(kc == n_sc - 1),
                            )
                        ctx = ctxp.tile([128, qb], F32R, tag="ctx")
                        nc.vector.tensor_mul(ctx[:, :], psa[:, :], recip[:, :])
                        nc.sync.dma_start(out=ctxT_d[fsl, qsl], in_=ctx[:, :])

            ctx_r = ctxT_d.rearrange("(fc p) t -> p fc t", p=128)
            wo_r = woT.rearrange("(fc p) hh -> p fc hh", p=128)
            with (
                tc.tile_pool(name="ctx3", bufs=1) as ctx3p,
                tc.tile_pool(name="wo", bufs=2) as wop,
                tc.tile_pool(name="ps_o", bufs=8, space="PSUM") as ps_o,
                tc.tile_pool(name="ev_o", bufs=8) as evo,
            ):
                ctx3 = ctx3p.tile([128, n_fc, s], F32R, tag="ctx3")
                nc.sync.dma_start(out=ctx3[:, :, :], in_=ctx_r[:, :, :])
                for b in range(n_hb):
                    hsl = slice(b * hb, (b + 1) * hb)
                    wo = wop.tile([128, n_fc, hb], F32R, tag="wo")
                    nc.sync.dma_start(out=wo[:, :, :], in_=wo_r[:, :, hsl])
                    for tg in range(0, n_sc, 8):
                        pss = [
                            ps_o.tile([128, hb], F32, tag="ps_o", name="ps_o")
                            for _ in range(min(8, n_sc - tg))
                        ]
                        for fc in range(n_fc):
                            for i, ps in enumerate(pss):
                                t = tg + i
                                nc.tensor.matmul(
                                    ps[:, :],
                                    r(ctx3[:, fc, t * 128 : (t + 1) * 128]),
                                    r(wo[:, fc, :]),
                                    start=(fc == 0),
                                    stop=(fc == n_fc - 1),
                                )
                        for i, ps in enumerate(pss):
                            t = tg + i
                            ob = evo.tile([128, hb], F32, tag="ob")
                            nc.scalar.copy(ob[:, :], ps[:, :])
                            nc.sync.dma_start(
                                out=out[t * 128 : (t + 1) * 128, hsl],
                                in_=ob[:, :],
                            )

    nc.compile()
    return nc


def rope_tables(s=S):
    inv = 1.0 / (ROPE_BASE ** (np.arange(0, D, 2, dtype=np.float32) / D))
    t = np.arange(s, dtype=np.float32)
    freqs = t[:, None] * inv[None, :]
    emb = np.concatenate([freqs, freqs], axis=-1)
    return (
        np.ascontiguousarray(np.cos(emb).T.astype(np.float32)),
        np.ascontiguousarray(np.sin(emb).T.astype(np.float32)),
    )


def perm_mat():
    p = np.zeros((D, D), np.float32)
    for d in range(D // 2):
        p[d + D // 2, d] = -1.0
        p[d, d + D // 2] = 1.0
    return p


def make_in_maps(hidden_states, Wq, Wk, Wv, Wo, s=S, f=F, dp=DP, tp=TP):
    cosT, sinT = rope_tables(s)
    pm = perm_mat()
    xTs = [np.ascontiguousarray(hidden_states[b].T) for b in range(dp)]
    WqT, WkT, WvT, WoT = Wq.T, Wk.T, Wv.T, Wo.T
    in_maps = []
    for c in range(dp * tp):
        b, g = divmod(c, tp)
        fsl = slice(g * f, (g + 1) * f)
        in_maps.append({
            "xT": xTs[b],
            "wqT": np.ascontiguousarray(WqT[:, fsl]),
            "wkT": np.ascontiguousarray(WkT[:, fsl]),
            "wvT": np.ascontiguousarray(WvT[:, fsl]),
            "woT": np.ascontiguousarray(WoT[fsl, :]),
            "cosT": cosT,
            "sinT": sinT,
            "pmat": pm,
        })
    return in_maps


_NC_CACHE = {}
TRACE = False
LAST_RESULT = None


def kernel(hidden_states, Wq, Wk, Wv, Wo):
    global LAST_RESULT
    hidden_states = np.asarray(hidden_states, np.float32)
    Wq, Wk, Wv, Wo = (np.asarray(w, np.float32) for w in (Wq, Wk, Wv, Wo))
    if "nc" not in _NC_CACHE:
        _NC_CACHE["nc"] = build_nc()
    nc = _NC_CACHE["nc"]
    in_maps = make_in_maps(hidden_states, Wq, Wk, Wv, Wo)
    res = run_bass_kernel_spmd(
        nc, in_maps, core_ids=list(range(N_CORES)), trace=TRACE
    )
    LAST_RESULT = res
    parts = [res.results[c]["out"] for c in range(N_CORES)]
    out = np.stack([
        parts[0] + parts[1] + parts[2] + parts[3],
        parts[4] + parts[5] + parts[6] + parts[7],
    ]).astype(np.float32)
    return out
